# revision 9
# baseline (speedup 1.0000x reference)
"""GraphConv (DGL norm='both') + log_softmax on 8 Trainium2 NeuronCores.

Strategy: source-sharded scatter-add (no per-edge gather).
  Launch 1 (per core): project its 12500 local src nodes
  m = (h @ W) * out_deg^-1/2 (bf16, SBUF-resident, in per-quarter
  degree-sorted order), then dma_scatter_add m rows directly into 4 local
  HBM quarter-tables of destination partial sums. Scatter calls are
  organized as (quarter, round r) passes: pass r adds "the r-th edge" of
  every src that still has edges, so the addend is simply a prefix of the
  degree-sorted m table read in place - no gather, no expansion. A host-side
  per-round matching picks WHICH edge each src contributes so that
  destination rows are distinct within each call (the DMA scatter-add
  drops duplicate rows within one call).
  Host: repack the 8 cores' partial tables per destination core (pure
  slicing/transpose, like the baseline's m concat).
  Launch 2 (per core): sum its 8 partial slices, apply in_deg norm + bias +
  log_softmax, write out.

Degrees and edge metadata (sort orders, per-call index arrays) are
sharding-prep computed on the host (numpy); all FLOPs on h/W/b/m
(projection, normalization, aggregation, softmax) run on device.
"""

import contextlib

import numpy as np
import ml_dtypes

import concourse.bass as bass
import concourse.bacc as bacc
import concourse.mybir as mybir
import concourse.tile as tile
from concourse.bass_utils import run_bass_kernel_spmd

P = 128
N_NODES = 100000
N_EDGES = 3200000
IN_DIM = 256
OUT_DIM = 64
NCORES = 8
G = N_NODES // NCORES          # 12500 src/dst nodes per core
NG = (G + P - 1) // P          # 98 chunks of 128
GPAD = NG * P                  # 12544
NQ = 4                         # dst quarter tables (int16 index range)
QROWS = 2 * GPAD               # 25088 real rows per quarter (2 dst cores)
TROWS = QROWS + 1              # +1 dump row
DUMP = QROWS
SUB = 48 * P                   # 6144 idx per scatter call (desc-ring limit)
HBLK = 16                      # h chunks per DMA block in projection

_f32 = mybir.dt.float32
_bf16 = mybir.dt.bfloat16
_i16 = mybir.dt.int16


# ---------------------------------------------------------------- launch 1
def build_launch_1(plan, repeat=1):
    """plan: dict with
      calls: list of (q, c0_chunk, ncols_chunks, num_idxs, idx_off16)
      tot_idx16: total idx columns ([128, tot_idx16] int16 tensor)
    """
    nc = bacc.Bacc("TRN2", target_bir_lowering=False, debug=False,
                   num_devices=NCORES)
    hT = nc.dram_tensor("hT", [NQ, 2, P, GPAD], _bf16, kind="ExternalInput")
    W = nc.dram_tensor("W", [IN_DIM, OUT_DIM], _bf16, kind="ExternalInput")
    odeg = nc.dram_tensor("odeg", [NQ, P, NG], _f32, kind="ExternalInput")
    gidx = nc.dram_tensor("gidx", [P, plan["tot_idx16"]], _i16,
                          kind="ExternalInput")
    tabs = [nc.dram_tensor(f"t{q}", [TROWS, P], _bf16, kind="ExternalOutput")
            for q in range(NQ)]

    with tile.TileContext(nc) as tc:
        loop = tc.For_i(0, repeat, 1) if repeat > 1 \
            else contextlib.nullcontext()
        with loop, \
                tc.tile_pool(name="const", bufs=1) as cpool, \
                tc.tile_pool(name="hblk", bufs=3) as hpool, \
                tc.tile_pool(name="idx", bufs=4) as ipool, \
                tc.tile_pool(name="psum", bufs=4, space="PSUM") as psum:
            w0 = cpool.tile([P, OUT_DIM], _bf16, tag="w0")
            w1 = cpool.tile([P, OUT_DIM], _bf16, tag="w1")
            nc.sync.dma_start(out=w0[:], in_=W[0:P, :])
            nc.sync.dma_start(out=w1[:], in_=W[P:2 * P, :])

            dt_ = cpool.tile([P, NQ * NG], _f32, tag="deg")
            norm = cpool.tile([P, NQ * NG], _f32, tag="norm")
            for q in range(NQ):
                nc.sync.dma_start(out=dt_[:, q * NG:(q + 1) * NG],
                                  in_=odeg[q, :, :])
            nc.vector.tensor_scalar_max(out=dt_[:], in0=dt_[:], scalar1=1.0)
            nc.vector.reciprocal(out=dt_[:], in_=dt_[:])
            nc.scalar.sqrt(out=norm[:], in_=dt_[:])

            # projection into 4 per-quarter sorted m tables (bf16, SBUF)
            mq = [cpool.tile([P, NG, OUT_DIM], _bf16, tag=f"m{q}",
                             name=f"mq{q}")
                  for q in range(NQ)]
            for q in range(NQ):
                for g0 in range(0, NG, HBLK):
                    nb = min(HBLK, NG - g0)
                    l0 = hpool.tile([P, HBLK * P], _bf16, tag="l0")
                    l1 = hpool.tile([P, HBLK * P], _bf16, tag="l1")
                    nc.sync.dma_start(out=l0[:, :nb * P],
                                      in_=hT[q, 0, :, g0 * P:(g0 + nb) * P])
                    nc.sync.dma_start(out=l1[:, :nb * P],
                                      in_=hT[q, 1, :, g0 * P:(g0 + nb) * P])
                    for j in range(nb):
                        g = g0 + j
                        acc = psum.tile([P, OUT_DIM], _f32, tag="acc")
                        nc.tensor.matmul(acc[:], l0[:, j * P:(j + 1) * P],
                                         w0[:], start=True, stop=False)
                        nc.tensor.matmul(acc[:], l1[:, j * P:(j + 1) * P],
                                         w1[:], start=False, stop=True)
                        nc.scalar.activation(
                            out=mq[q][:, g, :], in_=acc[:],
                            func=mybir.ActivationFunctionType.Identity,
                            scale=norm[:, q * NG + g:q * NG + g + 1])

            # scatter-add passes
            for (q, c0, ncc, nidx, off16) in plan["calls"]:
                n16 = (nidx + 15) // 16
                ix = ipool.tile([P, n16], _i16, tag="ix")
                nc.sync.dma_start(out=ix[:], in_=gidx[:, off16:off16 + n16])
                nc.gpsimd.dma_scatter_add(
                    out_ap=tabs[q][:, 0:OUT_DIM],
                    in_ap=mq[q][:, c0:c0 + ncc, :],
                    idxs_ap=ix[:],
                    num_idxs=nidx,
                    num_idxs_reg=nidx,
                    elem_size=OUT_DIM,
                    elem_step=P,
                    single_packet=False,
                )
    nc.compile()
    return nc


# ---------------------------------------------------------------- launch 2
def build_launch_2(repeat=1):
    nc = bacc.Bacc("TRN2", target_bir_lowering=False, debug=False,
                   num_devices=NCORES)
    pp = nc.dram_tensor("pp", [NCORES, P, NG * OUT_DIM], _bf16,
                        kind="ExternalInput")
    ideg = nc.dram_tensor("ideg", [P, NG], _f32, kind="ExternalInput")
    brep = nc.dram_tensor("brep", [P, OUT_DIM], _f32, kind="ExternalInput")
    out = nc.dram_tensor("out", [P, NG * OUT_DIM], _f32,
                         kind="ExternalOutput")

    with tile.TileContext(nc) as tc:
        loop = tc.For_i(0, repeat, 1) if repeat > 1 \
            else contextlib.nullcontext()
        with loop, \
                tc.tile_pool(name="const", bufs=1) as cpool, \
                tc.tile_pool(name="work", bufs=4) as pool:
            bt = cpool.tile([P, OUT_DIM], _f32, tag="b")
            nc.sync.dma_start(out=bt[:], in_=brep[:, :])

            dt_ = cpool.tile([P, NG], _f32, tag="deg")
            norm = cpool.tile([P, NG], _f32, tag="norm")
            nc.sync.dma_start(out=dt_[:], in_=ideg[:, :])
            nc.vector.tensor_scalar_max(out=dt_[:], in0=dt_[:], scalar1=1.0)
            nc.vector.reciprocal(out=dt_[:], in_=dt_[:])
            nc.scalar.sqrt(out=norm[:], in_=dt_[:])

            # stream partial tiles; accumulate in f32
            acc = cpool.tile([P, NG * OUT_DIM], _f32, tag="acc")
            pt0 = pool.tile([P, NG * OUT_DIM], _bf16, tag="pt")
            pt1 = pool.tile([P, NG * OUT_DIM], _bf16, tag="pt")
            nc.sync.dma_start(out=pt0[:], in_=pp[0, :, :])
            nc.sync.dma_start(out=pt1[:], in_=pp[1, :, :])
            nc.vector.tensor_add(out=acc[:], in0=pt0[:], in1=pt1[:])
            for c in range(2, NCORES):
                ptc = pool.tile([P, NG * OUT_DIM], _bf16, tag="pt")
                nc.sync.dma_start(out=ptc[:], in_=pp[c, :, :])
                nc.vector.tensor_add(out=acc[:], in0=acc[:], in1=ptc[:])

            y_all = cpool.tile([P, NG * OUT_DIM], _f32, tag="yall")
            s_all = cpool.tile([P, NG], _f32, tag="sall")
            for g in range(NG):
                x = pool.tile([P, OUT_DIM], _f32, tag="x")
                nc.vector.tensor_scalar_mul(
                    out=x[:], in0=acc[:, g * OUT_DIM:(g + 1) * OUT_DIM],
                    scalar1=norm[:, g:g + 1])
                nc.vector.tensor_add(out=x[:], in0=x[:], in1=bt[:])
                nmx = pool.tile([P, 1], _f32, tag="nmx")
                nc.vector.tensor_reduce(out=nmx[:], in_=x[:],
                                        axis=mybir.AxisListType.X,
                                        op=mybir.AluOpType.max,
                                        negate=True)
                nc.vector.tensor_scalar_add(
                    out=y_all[:, g * OUT_DIM:(g + 1) * OUT_DIM],
                    in0=x[:], scalar1=nmx[:, :1])
                e = pool.tile([P, OUT_DIM], _f32, tag="e")
                nc.scalar.activation(
                    out=e[:], in_=x[:],
                    func=mybir.ActivationFunctionType.Exp,
                    bias=nmx[:, :1], accum_out=s_all[:, g:g + 1])

            ls_all = cpool.tile([P, NG], _f32, tag="lsall")
            nc.scalar.activation(out=ls_all[:], in_=s_all[:],
                                 func=mybir.ActivationFunctionType.Ln)
            fin = cpool.tile([P, NG * OUT_DIM], _f32, tag="fin")
            for g in range(NG):
                nc.vector.tensor_scalar_sub(
                    out=fin[:, g * OUT_DIM:(g + 1) * OUT_DIM],
                    in0=y_all[:, g * OUT_DIM:(g + 1) * OUT_DIM],
                    scalar1=ls_all[:, g:g + 1])
            nc.sync.dma_start(out=out[:, :], in_=fin[:])
    nc.compile()
    return nc


# ------------------------------------------------------------- host prep
def _wrap16(flat):
    """int16 idx list (len % 16 == 0) -> [128, len/16], replicated x8."""
    s = len(flat) // 16
    blk = flat.reshape(s, 16).T
    arr = np.empty((P, s), dtype=np.int16)
    for grp in range(8):
        arr[grp * 16:(grp + 1) * 16, :] = blk
    return arr


def _assign_range(ps, dst, starts, cur, rem, chosen):
    """Place one edge for as many positions in ps as possible, distinct dsts.

    Vectorized greedy waves (remaining-ascending priority) followed by Kuhn
    augmentation for the losers. Consumes chosen edges (swap-to-cursor)
    and fills chosen[p] with the dst. Returns # placed.
    """
    ps = ps[np.argsort(rem[ps], kind="stable")]
    owner = np.full(TROWS, -1, dtype=np.int64)   # dst row -> pos
    pick = np.full(GPAD, -1, dtype=np.int64)     # pos -> abs edge index
    pending = ps
    t = np.zeros(len(ps), dtype=np.int64)
    while len(pending):
        ok = t < rem[pending]
        pending = pending[ok]
        t = t[ok]
        if not len(pending):
            break
        ei = starts[pending] + cur[pending] + t
        cand = dst[ei]
        free = owner[cand] < 0
        _, first = np.unique(cand, return_index=True)
        isf = np.zeros(len(cand), dtype=bool)
        isf[first] = True
        win = free & isf
        owner[cand[win]] = pending[win]
        pick[pending[win]] = ei[win]
        pending = pending[~win]
        t = t[~win] + 1

    losers = ps[pick[ps] < 0]
    k_next = int((rem > 1).sum())  # approx next-round prefix length

    def aug(p, visited):
        s = starts[p] + cur[p]
        for e in range(s, s + rem[p]):
            d = int(dst[e])
            if owner[d] < 0 and d not in visited:
                owner[d] = p
                pick[p] = e
                return True
        for e in range(s, s + rem[p]):
            d = int(dst[e])
            if d in visited:
                continue
            visited.add(d)
            if aug(int(owner[d]), visited):
                owner[d] = p
                pick[p] = e
                return True
        return False

    def victim(p):
        # no augmenting path: steal a dst from a flexible early-position
        # owner, which then harmlessly skips this round
        s = starts[p] + cur[p]
        best, bd, be = -1, -1, -1
        for e in range(s, s + rem[p]):
            d = int(dst[e])
            q = int(owner[d])
            if q >= 0 and q != p and rem[q] >= 2 and q < k_next:
                if best < 0 or rem[q] > rem[best]:
                    best, bd, be = q, d, e
        if best >= 0:
            owner[bd] = p
            pick[p] = be
            pick[best] = -1
            return True
        return False

    for p in losers:
        p = int(p)
        if not aug(p, set()):
            victim(p)

    wp = ps[pick[ps] >= 0]
    if not len(wp):
        return 0
    wei = pick[wp]
    chosen[wp] = dst[wei]
    we0 = starts[wp] + cur[wp]
    tmp = dst[wei].copy()
    dst[wei] = dst[we0]
    dst[we0] = tmp
    cur[wp] += 1
    rem[wp] -= 1
    return len(wp)


def _match_core_quarter(pos, qrow, npos, rng):
    """Assign edges (pos -> qrow) to rounds; distinct qrow per sub-call.

    Returns (rounds, leftover): rounds = list of np arrays `chosen` of
    length n_r (prefix slots; -1 = dump), leftover = (pos, dst) arrays of
    unplaced straggler edges.
    """
    order = np.argsort(pos, kind="stable")
    dst = qrow[order].astype(np.int64).copy()
    deg = np.bincount(pos, minlength=npos)
    starts = np.zeros(npos + 1, dtype=np.int64)
    np.cumsum(deg, out=starts[1:])
    keys = rng.random(len(dst)) + np.repeat(np.arange(npos), deg)
    dst = dst[np.argsort(keys, kind="stable")]

    rem = deg.copy()
    cur = np.zeros(npos, dtype=np.int64)
    rounds = []
    guard = 0
    while rem.max() > 0 and guard < 48:
        guard += 1
        act = np.nonzero(rem > 0)[0]
        # cap prefix: stragglers far beyond the dense prefix are skipped
        n_r = min(int(act[-1]) + 1, len(act) + 192)
        ps_all = act[act < n_r]
        if len(ps_all) == 0:
            break
        n_r = int(ps_all[-1]) + 1
        chosen = np.full(n_r, -1, dtype=np.int64)
        placed = 0
        for lo in range(0, n_r, SUB):
            hi = min(lo + SUB, n_r)
            ps = ps_all[(ps_all >= lo) & (ps_all < hi)]
            if len(ps):
                placed += _assign_range(ps, dst, starts, cur, rem, chosen)
        if placed == 0:
            break
        rounds.append(chosen)
    # leftovers
    lp, ld = [], []
    for p in np.nonzero(rem > 0)[0]:
        s = starts[p] + cur[p]
        for e in range(int(rem[p])):
            lp.append(int(p))
            ld.append(int(dst[s + e]))
    return rounds, (np.array(lp, dtype=np.int64),
                    np.array(ld, dtype=np.int64))


def prepare(h, W, b, edges):
    h = np.asarray(h, dtype=np.float32)
    W = np.asarray(W, dtype=np.float32)
    b = np.asarray(b, dtype=np.float32)
    src = np.asarray(edges[0], dtype=np.int64)
    dst = np.asarray(edges[1], dtype=np.int64)

    out_deg = np.bincount(src, minlength=N_NODES).astype(np.float32)
    in_deg = np.bincount(dst, minlength=N_NODES).astype(np.float32)

    # destination padded-global row + quarter
    dcore = dst // G
    drow = dcore * GPAD + (dst - dcore * G)
    dq = drow // QROWS
    dqrow = drow - dq * QROWS

    score = src // G
    rng = np.random.default_rng(12345)

    # per (core, quarter) matchings
    per_core = []
    for c in range(NCORES):
        msk = score == c
        sloc = (src[msk] - c * G).astype(np.int64)
        q_e = dq[msk]
        qr_e = dqrow[msk]
        deg_q = np.zeros((NQ, G), dtype=np.int64)
        orders = []
        rounds_q = []
        left_q = []
        for q in range(NQ):
            mq = q_e == q
            deg_q[q] = np.bincount(sloc[mq], minlength=G)
            order = np.argsort(-deg_q[q], kind="stable")  # degree desc
            orders.append(order)
            inv = np.empty(G, dtype=np.int64)
            inv[order] = np.arange(G)
            pos = inv[sloc[mq]]
            rounds, left = _match_core_quarter(pos, qr_e[mq], GPAD, rng)
            rounds_q.append(rounds)
            left_q.append(left)
        per_core.append(dict(deg_q=deg_q, orders=orders, rounds=rounds_q,
                             left=left_q))

    # cleanup rounds for leftover straggler edges: chunk-range calls whose
    # ranges are unioned across cores (static SPMD structure)
    cleanup = []  # list of (q, c0, ncc) -> per-core chosen appended to rounds
    for _ in range(24):
        anyleft = False
        for q in range(NQ):
            chunks = set()
            for pc in per_core:
                lp, ld = pc["left"][q]
                if len(lp):
                    anyleft = True
                    chunks.update((lp // P).tolist())
            if not chunks:
                continue
            chs = sorted(chunks)
            ranges = []
            c0 = cprev = chs[0]
            for ch in chs[1:]:
                if ch - cprev > 2 or ch - c0 >= SUB // P:
                    ranges.append((c0, cprev))
                    c0 = ch
                cprev = ch
            ranges.append((c0, cprev))
            for (ra, rb) in ranges:
                ncc = rb - ra + 1
                cleanup.append((q, ra, ncc))
                for pc in per_core:
                    lp, ld = pc["left"][q]
                    chosen = np.full(ncc * P, -1, dtype=np.int64)
                    used = set()
                    keep = np.ones(len(lp), dtype=bool)
                    placed_pos = set()
                    for i in range(len(lp)):
                        p, d = int(lp[i]), int(ld[i])
                        rel = p - ra * P
                        if 0 <= rel < ncc * P and p not in placed_pos \
                                and d not in used:
                            chosen[rel] = d
                            used.add(d)
                            placed_pos.add(p)
                            keep[i] = False
                    pc["left"][q] = (lp[keep], ld[keep])
                    pc.setdefault("cleanup", []).append(chosen)
        if not anyleft:
            break
    for pc in per_core:
        for q in range(NQ):
            assert len(pc["left"][q][0]) == 0, "cleanup did not converge"

    # static call plan: per (q, r): n = max over cores; split into SUB-sized
    # calls; interleave across quarters so WAW chains on the 4 tables overlap
    per_q_calls = {q: [] for q in range(NQ)}   # (c0, ncc, n, fill_spec)
    for q in range(NQ):
        rmax = max(len(pc["rounds"][q]) for pc in per_core)
        for r in range(rmax):
            n_qr = max((len(pc["rounds"][q][r])
                        if r < len(pc["rounds"][q]) else 0)
                       for pc in per_core)
            if n_qr == 0:
                continue
            for lo in range(0, n_qr, SUB):
                n_sub = min(SUB, n_qr - lo)
                per_q_calls[q].append(
                    (lo // P, (n_sub + P - 1) // P, n_sub, ("r", r, lo)))
    ci = 0
    for (q, ra, ncc) in cleanup:
        per_q_calls[q].append((ra, ncc, ncc * P, ("c", ci, 0)))
        ci += 1
    # round-robin across quarters
    plan_calls = []
    call_fill = []
    idxs = [0] * NQ
    off16 = 0
    while any(idxs[q] < len(per_q_calls[q]) for q in range(NQ)):
        for q in range(NQ):
            if idxs[q] >= len(per_q_calls[q]):
                continue
            (c0, ncc, n_sub, spec) = per_q_calls[q][idxs[q]]
            idxs[q] += 1
            n_sub16 = ((n_sub + 15) // 16) * 16
            plan_calls.append((q, c0, ncc, n_sub, off16))
            call_fill.append((q, spec, n_sub, n_sub16))
            off16 += n_sub16 // 16
    plan = dict(calls=plan_calls, tot_idx16=off16)

    # per-core gidx arrays
    gidx_cores = []
    stats_slots = 0
    stats_real = 0
    for c in range(NCORES):
        pc = per_core[c]
        cleanup_list = pc.get("cleanup", [])
        flat = np.full(off16 * 16, DUMP, dtype=np.int16)
        pos16 = 0
        for (q, spec, n_sub, n_sub16) in call_fill:
            seg = np.full(n_sub16, DUMP, dtype=np.int16)
            kind, a, lo = spec
            if kind == "r":
                rounds = pc["rounds"][q]
                ch = rounds[a] if a < len(rounds) else None
            else:
                ch = cleanup_list[a] if a < len(cleanup_list) else None
                lo = 0
            if ch is not None:
                npart = min(max(len(ch) - lo, 0), n_sub)
                if npart > 0:
                    part = ch[lo:lo + npart]
                    seg[:npart] = np.where(part >= 0, part,
                                           DUMP).astype(np.int16)
                    stats_real += int((part >= 0).sum())
            if c == 0:
                stats_slots += n_sub
            flat[pos16 * 16:pos16 * 16 + n_sub16] = seg
            pos16 += n_sub16 // 16
        gidx_cores.append(_wrap16(flat))
    plan["stats"] = (stats_real, stats_slots)

    # hT per (core, quarter): degree-sorted, padded, bf16, [2, 128, GPAD]
    hT_cores = []
    odeg_cores = []
    for c in range(NCORES):
        pc = per_core[c]
        hts = np.zeros((NQ, 2, P, GPAD), dtype=ml_dtypes.bfloat16)
        odt = np.zeros((NQ, P, NG), dtype=np.float32)
        hl = h[c * G:(c + 1) * G]          # [12500, 256]
        odl = out_deg[c * G:(c + 1) * G]
        for q in range(NQ):
            order = pc["orders"][q]
            hs = np.zeros((GPAD, IN_DIM), dtype=np.float32)
            hs[:G] = hl[order]
            hts[q] = np.ascontiguousarray(
                hs.T.reshape(2, P, GPAD)).astype(ml_dtypes.bfloat16)
            od = np.ones(GPAD, dtype=np.float32)
            od[:G] = odl[order]
            odt[q] = od.reshape(NG, P).T
        hT_cores.append(hts)
        odeg_cores.append(odt)

    # L2 tiles
    ideg_cores = []
    for c in range(NCORES):
        d = np.ones(GPAD, dtype=np.float32)
        d[:G] = in_deg[c * G:(c + 1) * G]
        ideg_cores.append(np.ascontiguousarray(d.reshape(NG, P).T))
    brep = np.broadcast_to(b, (P, OUT_DIM)).astype(np.float32).copy()

    return dict(plan=plan, gidx=gidx_cores, hT=hT_cores, odeg=odeg_cores,
                ideg=ideg_cores, brep=brep,
                W=W.astype(ml_dtypes.bfloat16))


_cache = {}


def _get_programs(plan):
    if "a" not in _cache:
        _cache["a"] = build_launch_1(plan)
    if "b" not in _cache:
        _cache["b"] = build_launch_2()
    return _cache["a"], _cache["b"]


def run_launch_1(nc_a, prep):
    in_maps = [{"hT": prep["hT"][c], "W": prep["W"],
                "odeg": prep["odeg"][c], "gidx": prep["gidx"][c]}
               for c in range(NCORES)]
    res = run_bass_kernel_spmd(nc_a, in_maps, list(range(NCORES)))
    return [[np.asarray(r[f"t{q}"]) for q in range(NQ)]
            for r in res.results]


def repack_partials(ptabs):
    """ptabs[c][q]: [TROWS, 128] bf16 -> pp[r]: [8, 128, NG*64] bf16."""
    pps = []
    for r in range(NCORES):
        q, half = r // 2, r % 2
        sl = np.empty((NCORES, P, NG * OUT_DIM), dtype=ml_dtypes.bfloat16)
        for c in range(NCORES):
            t = ptabs[c][q][half * GPAD:(half + 1) * GPAD, 0:OUT_DIM]
            sl[c] = t.reshape(NG, P, OUT_DIM).transpose(1, 0, 2).reshape(
                P, NG * OUT_DIM)
        pps.append(sl)
    return pps


def run_launch_2(nc_b, prep, pps):
    in_maps = [dict(pp=pps[c], ideg=prep["ideg"][c], brep=prep["brep"])
               for c in range(NCORES)]
    res = run_bass_kernel_spmd(nc_b, in_maps, list(range(NCORES)))
    outs = []
    for c in range(NCORES):
        o = np.asarray(res.results[c]["out"]).reshape(P, NG, OUT_DIM)
        outs.append(o.transpose(1, 0, 2).reshape(GPAD, OUT_DIM)[:G])
    return np.concatenate(outs, axis=0)


def kernel(h, W, b, edges):
    prep = prepare(h, W, b, edges)
    nc_a, nc_b = _get_programs(prep["plan"])
    ptabs = run_launch_1(nc_a, prep)
    pps = repack_partials(ptabs)
    out = run_launch_2(nc_b, prep, pps)
    return out.astype(np.float32)


# revision 21
# speedup vs baseline: 1.0860x; 1.0860x over previous
"""GraphConv (DGL norm='both') + log_softmax on 8 Trainium2 NeuronCores.

Strategy: source-sharded scatter-add (no per-edge gather).
  Launch 1 (per core): project its 12500 local src nodes
  m = (h @ W) * out_deg^-1/2 (bf16, SBUF-resident, in per-quarter
  degree-sorted order), then dma_scatter_add m rows directly into 4 local
  HBM quarter-tables of destination partial sums. Scatter calls are
  organized as (quarter, round r) passes: pass r adds "the r-th edge" of
  every src that still has edges, so the addend is simply a prefix of the
  degree-sorted m table read in place - no gather, no expansion. A host-side
  per-round matching picks WHICH edge each src contributes so that
  destination rows are distinct within each call (the DMA scatter-add
  drops duplicate rows within one call).
  Host: repack the 8 cores' partial tables per destination core (pure
  slicing/transpose, like the baseline's m concat).
  Launch 2 (per core): sum its 8 partial slices, apply in_deg norm + bias +
  log_softmax, write out.

Degrees and edge metadata (sort orders, per-call index arrays) are
sharding-prep computed on the host (numpy); all FLOPs on h/W/b/m
(projection, normalization, aggregation, softmax) run on device.
"""

import contextlib

import numpy as np
import ml_dtypes

import concourse.bass as bass
import concourse.bacc as bacc
import concourse.mybir as mybir
import concourse.tile as tile
from concourse.bass_utils import run_bass_kernel_spmd

P = 128
N_NODES = 100000
N_EDGES = 3200000
IN_DIM = 256
OUT_DIM = 64
NCORES = 8
G = N_NODES // NCORES          # 12500 src/dst nodes per core
NG = (G + P - 1) // P          # 98 chunks of 128
GPAD = NG * P                  # 12544
NQ = 4                         # dst quarter tables (int16 index range)
QROWS = 2 * GPAD               # 25088 real rows per quarter (2 dst cores)
TROWS = QROWS + 1              # +1 dump row
DUMP = QROWS
SUB = 63 * P                   # 8064 idx per scatter call (desc-ring limit)
HBLK = 16                      # h chunks per DMA block in projection

_f32 = mybir.dt.float32
_bf16 = mybir.dt.bfloat16
_i16 = mybir.dt.int16


# ---------------------------------------------------------------- launch 1
def build_launch_1(plan, repeat=1):
    """plan: dict with
      calls: list of (q, c0_chunk, ncols_chunks, num_idxs, idx_off16)
      tot_idx16: total idx columns ([128, tot_idx16] int16 tensor)
    """
    nc = bacc.Bacc("TRN2", target_bir_lowering=False, debug=False,
                   num_devices=NCORES)
    hT = nc.dram_tensor("hT", [NQ, 2, P, GPAD], _bf16, kind="ExternalInput")
    W = nc.dram_tensor("W", [IN_DIM, OUT_DIM], _bf16, kind="ExternalInput")
    odeg = nc.dram_tensor("odeg", [NQ, P, NG], _f32, kind="ExternalInput")
    gidx = nc.dram_tensor("gidx", [P, plan["tot_idx16"]], _i16,
                          kind="ExternalInput")
    tabs = [nc.dram_tensor(f"t{q}", [TROWS, P], _bf16, kind="ExternalOutput")
            for q in range(NQ)]

    with tile.TileContext(nc) as tc:
        loop = tc.For_i(0, repeat, 1) if repeat > 1 \
            else contextlib.nullcontext()
        with loop, \
                tc.tile_pool(name="const", bufs=1) as cpool, \
                tc.tile_pool(name="hblk", bufs=3) as hpool, \
                tc.tile_pool(name="idx", bufs=8) as ipool, \
                tc.tile_pool(name="psum", bufs=4, space="PSUM") as psum:
            w0 = cpool.tile([P, OUT_DIM], _bf16, tag="w0")
            w1 = cpool.tile([P, OUT_DIM], _bf16, tag="w1")
            nc.sync.dma_start(out=w0[:], in_=W[0:P, :])
            nc.sync.dma_start(out=w1[:], in_=W[P:2 * P, :])

            dt_ = cpool.tile([P, NQ * NG], _f32, tag="deg")
            norm = cpool.tile([P, NQ * NG], _f32, tag="norm")
            for q in range(NQ):
                nc.sync.dma_start(out=dt_[:, q * NG:(q + 1) * NG],
                                  in_=odeg[q, :, :])
            nc.vector.tensor_scalar_max(out=dt_[:], in0=dt_[:], scalar1=1.0)
            nc.vector.reciprocal(out=dt_[:], in_=dt_[:])
            nc.scalar.sqrt(out=norm[:], in_=dt_[:])

            # projection into 4 per-quarter sorted m tables (bf16, SBUF)
            mq = [cpool.tile([P, NG, OUT_DIM], _bf16, tag=f"m{q}",
                             name=f"mq{q}")
                  for q in range(NQ)]
            for q in range(NQ):
                for g0 in range(0, NG, HBLK):
                    nb = min(HBLK, NG - g0)
                    l0 = hpool.tile([P, HBLK * P], _bf16, tag="l0")
                    l1 = hpool.tile([P, HBLK * P], _bf16, tag="l1")
                    nc.sync.dma_start(out=l0[:, :nb * P],
                                      in_=hT[q, 0, :, g0 * P:(g0 + nb) * P])
                    nc.sync.dma_start(out=l1[:, :nb * P],
                                      in_=hT[q, 1, :, g0 * P:(g0 + nb) * P])
                    for j in range(nb):
                        g = g0 + j
                        acc = psum.tile([P, OUT_DIM], _f32, tag="acc")
                        nc.tensor.matmul(acc[:], l0[:, j * P:(j + 1) * P],
                                         w0[:], start=True, stop=False)
                        nc.tensor.matmul(acc[:], l1[:, j * P:(j + 1) * P],
                                         w1[:], start=False, stop=True)
                        nc.scalar.activation(
                            out=mq[q][:, g, :], in_=acc[:],
                            func=mybir.ActivationFunctionType.Identity,
                            scale=norm[:, q * NG + g:q * NG + g + 1])

            # scatter-add passes; chain same-table calls (the DMA engines
            # lose updates if two in-flight calls RMW the same table row)
            from bass_rust import add_dep_helper
            last_q = [None] * NQ
            for (q, c0, ncc, nidx, off16) in plan["calls"]:
                n16 = (nidx + 15) // 16
                ix = ipool.tile([P, n16], _i16, tag="ix")
                nc.sync.dma_start(out=ix[:], in_=gidx[:, off16:off16 + n16])
                sc = nc.gpsimd.dma_scatter_add(
                    out_ap=tabs[q][:, 0:OUT_DIM],
                    in_ap=mq[q][:, c0:c0 + ncc, :],
                    idxs_ap=ix[:],
                    num_idxs=nidx,
                    num_idxs_reg=nidx,
                    elem_size=OUT_DIM,
                    elem_step=P,
                    single_packet=False,
                )
                if last_q[q] is not None:
                    add_dep_helper(sc.ins, last_q[q].ins,
                                   reason="same-table scatter WAW order")
                last_q[q] = sc
    nc.compile()
    return nc


# ---------------------------------------------------------------- launch 2
def build_launch_2(repeat=1):
    nc = bacc.Bacc("TRN2", target_bir_lowering=False, debug=False,
                   num_devices=NCORES)
    pp = nc.dram_tensor("pp", [NCORES, P, NG * OUT_DIM], _bf16,
                        kind="ExternalInput")
    ideg = nc.dram_tensor("ideg", [P, NG], _f32, kind="ExternalInput")
    brep = nc.dram_tensor("brep", [P, OUT_DIM], _f32, kind="ExternalInput")
    out = nc.dram_tensor("out", [P, NG * OUT_DIM], _f32,
                         kind="ExternalOutput")

    with tile.TileContext(nc) as tc:
        loop = tc.For_i(0, repeat, 1) if repeat > 1 \
            else contextlib.nullcontext()
        with loop, \
                tc.tile_pool(name="const", bufs=1) as cpool, \
                tc.tile_pool(name="work", bufs=4) as pool:
            bt = cpool.tile([P, OUT_DIM], _f32, tag="b")
            nc.sync.dma_start(out=bt[:], in_=brep[:, :])

            dt_ = cpool.tile([P, NG], _f32, tag="deg")
            norm = cpool.tile([P, NG], _f32, tag="norm")
            nc.sync.dma_start(out=dt_[:], in_=ideg[:, :])
            nc.vector.tensor_scalar_max(out=dt_[:], in0=dt_[:], scalar1=1.0)
            nc.vector.reciprocal(out=dt_[:], in_=dt_[:])
            nc.scalar.sqrt(out=norm[:], in_=dt_[:])

            # stream partial tiles; accumulate in f32
            acc = cpool.tile([P, NG * OUT_DIM], _f32, tag="acc")
            pt0 = pool.tile([P, NG * OUT_DIM], _bf16, tag="pt")
            pt1 = pool.tile([P, NG * OUT_DIM], _bf16, tag="pt")
            nc.sync.dma_start(out=pt0[:], in_=pp[0, :, :])
            nc.sync.dma_start(out=pt1[:], in_=pp[1, :, :])
            nc.vector.tensor_add(out=acc[:], in0=pt0[:], in1=pt1[:])
            for c in range(2, NCORES):
                ptc = pool.tile([P, NG * OUT_DIM], _bf16, tag="pt")
                nc.sync.dma_start(out=ptc[:], in_=pp[c, :, :])
                nc.vector.tensor_add(out=acc[:], in0=acc[:], in1=ptc[:])

            y_all = cpool.tile([P, NG * OUT_DIM], _f32, tag="yall")
            s_all = cpool.tile([P, NG], _f32, tag="sall")
            for g in range(NG):
                x = pool.tile([P, OUT_DIM], _f32, tag="x")
                nc.vector.tensor_scalar_mul(
                    out=x[:], in0=acc[:, g * OUT_DIM:(g + 1) * OUT_DIM],
                    scalar1=norm[:, g:g + 1])
                nc.vector.tensor_add(out=x[:], in0=x[:], in1=bt[:])
                nmx = pool.tile([P, 1], _f32, tag="nmx")
                nc.vector.tensor_reduce(out=nmx[:], in_=x[:],
                                        axis=mybir.AxisListType.X,
                                        op=mybir.AluOpType.max,
                                        negate=True)
                nc.vector.tensor_scalar_add(
                    out=y_all[:, g * OUT_DIM:(g + 1) * OUT_DIM],
                    in0=x[:], scalar1=nmx[:, :1])
                e = pool.tile([P, OUT_DIM], _f32, tag="e")
                nc.scalar.activation(
                    out=e[:], in_=x[:],
                    func=mybir.ActivationFunctionType.Exp,
                    bias=nmx[:, :1], accum_out=s_all[:, g:g + 1])

            ls_all = cpool.tile([P, NG], _f32, tag="lsall")
            nc.scalar.activation(out=ls_all[:], in_=s_all[:],
                                 func=mybir.ActivationFunctionType.Ln)
            fin = cpool.tile([P, NG * OUT_DIM], _f32, tag="fin")
            for g in range(NG):
                nc.vector.tensor_scalar_sub(
                    out=fin[:, g * OUT_DIM:(g + 1) * OUT_DIM],
                    in0=y_all[:, g * OUT_DIM:(g + 1) * OUT_DIM],
                    scalar1=ls_all[:, g:g + 1])
            nc.sync.dma_start(out=out[:, :], in_=fin[:])
    nc.compile()
    return nc


# ------------------------------------------------------------- host prep
def _wrap16(flat):
    """int16 idx list (len % 16 == 0) -> [128, len/16], replicated x8."""
    s = len(flat) // 16
    blk = flat.reshape(s, 16).T
    arr = np.empty((P, s), dtype=np.int16)
    for grp in range(8):
        arr[grp * 16:(grp + 1) * 16, :] = blk
    return arr


def _assign_range(ps, dst, starts, cur, rem, chosen):
    """Place one edge for as many positions in ps as possible, distinct dsts.

    Vectorized greedy waves (remaining-ascending priority) followed by Kuhn
    augmentation for the losers. Consumes chosen edges (swap-to-cursor)
    and fills chosen[p] with the dst. Returns # placed.
    """
    ps = ps[np.argsort(rem[ps], kind="stable")]
    owner = np.full(TROWS, -1, dtype=np.int64)   # dst row -> pos
    pick = np.full(GPAD, -1, dtype=np.int64)     # pos -> abs edge index
    pending = ps
    t = np.zeros(len(ps), dtype=np.int64)
    while len(pending):
        ok = t < rem[pending]
        pending = pending[ok]
        t = t[ok]
        if not len(pending):
            break
        ei = starts[pending] + cur[pending] + t
        cand = dst[ei]
        free = owner[cand] < 0
        _, first = np.unique(cand, return_index=True)
        isf = np.zeros(len(cand), dtype=bool)
        isf[first] = True
        win = free & isf
        owner[cand[win]] = pending[win]
        pick[pending[win]] = ei[win]
        pending = pending[~win]
        t = t[~win] + 1

    losers = ps[pick[ps] < 0]
    k_next = int((rem > 1).sum())  # approx next-round prefix length

    def aug(p, visited):
        s = starts[p] + cur[p]
        for e in range(s, s + rem[p]):
            d = int(dst[e])
            if owner[d] < 0 and d not in visited:
                owner[d] = p
                pick[p] = e
                return True
        for e in range(s, s + rem[p]):
            d = int(dst[e])
            if d in visited:
                continue
            visited.add(d)
            if aug(int(owner[d]), visited):
                owner[d] = p
                pick[p] = e
                return True
        return False

    def victim(p):
        # no augmenting path: steal a dst from a flexible early-position
        # owner, which then harmlessly skips this round
        s = starts[p] + cur[p]
        best, bd, be = -1, -1, -1
        for e in range(s, s + rem[p]):
            d = int(dst[e])
            q = int(owner[d])
            if q >= 0 and q != p and rem[q] >= 2 and q < k_next:
                if best < 0 or rem[q] > rem[best]:
                    best, bd, be = q, d, e
        if best >= 0:
            owner[bd] = p
            pick[p] = be
            pick[best] = -1
            return True
        return False

    for p in losers:
        p = int(p)
        if not aug(p, set()):
            victim(p)

    wp = ps[pick[ps] >= 0]
    if not len(wp):
        return 0
    wei = pick[wp]
    chosen[wp] = dst[wei]
    we0 = starts[wp] + cur[wp]
    tmp = dst[wei].copy()
    dst[wei] = dst[we0]
    dst[we0] = tmp
    cur[wp] += 1
    rem[wp] -= 1
    return len(wp)


def _match_core_quarter(pos, qrow, npos, rng):
    """Assign edges (pos -> qrow) to rounds; distinct qrow per sub-call.

    Returns (rounds, leftover): rounds = list of np arrays `chosen` of
    length n_r (prefix slots; -1 = dump), leftover = (pos, dst) arrays of
    unplaced straggler edges.
    """
    order = np.argsort(pos, kind="stable")
    dst = qrow[order].astype(np.int64).copy()
    deg = np.bincount(pos, minlength=npos)
    starts = np.zeros(npos + 1, dtype=np.int64)
    np.cumsum(deg, out=starts[1:])
    keys = rng.random(len(dst)) + np.repeat(np.arange(npos), deg)
    dst = dst[np.argsort(keys, kind="stable")]

    rem = deg.copy()
    cur = np.zeros(npos, dtype=np.int64)
    rounds = []
    guard = 0
    while rem.max() > 0 and guard < 48:
        guard += 1
        act = np.nonzero(rem > 0)[0]
        # cap prefix: stragglers far beyond the dense prefix are skipped
        n_r = min(int(act[-1]) + 1, len(act) + 192)
        ps_all = act[act < n_r]
        if len(ps_all) == 0:
            break
        n_r = int(ps_all[-1]) + 1
        chosen = np.full(n_r, -1, dtype=np.int64)
        placed = 0
        for lo in range(0, n_r, SUB):
            hi = min(lo + SUB, n_r)
            ps = ps_all[(ps_all >= lo) & (ps_all < hi)]
            if len(ps):
                placed += _assign_range(ps, dst, starts, cur, rem, chosen)
        if placed == 0:
            break
        rounds.append(chosen)
    # leftovers
    lp, ld = [], []
    for p in np.nonzero(rem > 0)[0]:
        s = starts[p] + cur[p]
        for e in range(int(rem[p])):
            lp.append(int(p))
            ld.append(int(dst[s + e]))
    return rounds, (np.array(lp, dtype=np.int64),
                    np.array(ld, dtype=np.int64))


def prepare(h, W, b, edges):
    h = np.asarray(h, dtype=np.float32)
    W = np.asarray(W, dtype=np.float32)
    b = np.asarray(b, dtype=np.float32)
    src = np.asarray(edges[0], dtype=np.int64)
    dst = np.asarray(edges[1], dtype=np.int64)

    out_deg = np.bincount(src, minlength=N_NODES).astype(np.float32)
    in_deg = np.bincount(dst, minlength=N_NODES).astype(np.float32)

    # destination padded-global row + quarter
    dcore = dst // G
    drow = dcore * GPAD + (dst - dcore * G)
    dq = drow // QROWS
    dqrow = drow - dq * QROWS

    score = src // G
    rng = np.random.default_rng(12345)

    # per (core, quarter) matchings
    per_core = []
    for c in range(NCORES):
        msk = score == c
        sloc = (src[msk] - c * G).astype(np.int64)
        q_e = dq[msk]
        qr_e = dqrow[msk]
        deg_q = np.zeros((NQ, G), dtype=np.int64)
        orders = []
        rounds_q = []
        left_q = []
        for q in range(NQ):
            mq = q_e == q
            deg_q[q] = np.bincount(sloc[mq], minlength=G)
            order = np.argsort(-deg_q[q], kind="stable")  # degree desc
            orders.append(order)
            inv = np.empty(G, dtype=np.int64)
            inv[order] = np.arange(G)
            pos = inv[sloc[mq]]
            rounds, left = _match_core_quarter(pos, qr_e[mq], GPAD, rng)
            rounds_q.append(rounds)
            left_q.append(left)
        per_core.append(dict(deg_q=deg_q, orders=orders, rounds=rounds_q,
                             left=left_q))

    # cleanup rounds for leftover straggler edges: chunk-range calls whose
    # ranges are unioned across cores (static SPMD structure)
    cleanup = []  # list of (q, c0, ncc) -> per-core chosen appended to rounds
    for _ in range(24):
        anyleft = False
        for q in range(NQ):
            chunks = set()
            for pc in per_core:
                lp, ld = pc["left"][q]
                if len(lp):
                    anyleft = True
                    chunks.update((lp // P).tolist())
            if not chunks:
                continue
            chs = sorted(chunks)
            ranges = []
            c0 = cprev = chs[0]
            for ch in chs[1:]:
                if ch - cprev > 2 or ch - c0 >= SUB // P:
                    ranges.append((c0, cprev))
                    c0 = ch
                cprev = ch
            ranges.append((c0, cprev))
            for (ra, rb) in ranges:
                ncc = rb - ra + 1
                cleanup.append((q, ra, ncc))
                for pc in per_core:
                    lp, ld = pc["left"][q]
                    chosen = np.full(ncc * P, -1, dtype=np.int64)
                    used = set()
                    keep = np.ones(len(lp), dtype=bool)
                    placed_pos = set()
                    for i in range(len(lp)):
                        p, d = int(lp[i]), int(ld[i])
                        rel = p - ra * P
                        if 0 <= rel < ncc * P and p not in placed_pos \
                                and d not in used:
                            chosen[rel] = d
                            used.add(d)
                            placed_pos.add(p)
                            keep[i] = False
                    pc["left"][q] = (lp[keep], ld[keep])
                    pc.setdefault("cleanup", []).append(chosen)
        if not anyleft:
            break
    for pc in per_core:
        for q in range(NQ):
            assert len(pc["left"][q][0]) == 0, "cleanup did not converge"

    # static call plan: per (q, r): n = max over cores; split into SUB-sized
    # calls; interleave across quarters so WAW chains on the 4 tables overlap
    per_q_calls = {q: [] for q in range(NQ)}   # (c0, ncc, n, fill_spec)
    for q in range(NQ):
        rmax = max(len(pc["rounds"][q]) for pc in per_core)
        for r in range(rmax):
            n_qr = max((len(pc["rounds"][q][r])
                        if r < len(pc["rounds"][q]) else 0)
                       for pc in per_core)
            if n_qr == 0:
                continue
            for lo in range(0, n_qr, SUB):
                n_sub = min(SUB, n_qr - lo)
                per_q_calls[q].append(
                    (lo // P, (n_sub + P - 1) // P, n_sub, ("r", r, lo)))
    ci = 0
    for (q, ra, ncc) in cleanup:
        per_q_calls[q].append((ra, ncc, ncc * P, ("c", ci, 0)))
        ci += 1
    # round-robin across quarters
    plan_calls = []
    call_fill = []
    idxs = [0] * NQ
    off16 = 0
    while any(idxs[q] < len(per_q_calls[q]) for q in range(NQ)):
        for q in range(NQ):
            if idxs[q] >= len(per_q_calls[q]):
                continue
            (c0, ncc, n_sub, spec) = per_q_calls[q][idxs[q]]
            idxs[q] += 1
            n_sub16 = ((n_sub + 15) // 16) * 16
            plan_calls.append((q, c0, ncc, n_sub, off16))
            call_fill.append((q, spec, n_sub, n_sub16))
            off16 += n_sub16 // 16
    plan = dict(calls=plan_calls, tot_idx16=off16)

    # per-core gidx arrays
    gidx_cores = []
    stats_slots = 0
    stats_real = 0
    for c in range(NCORES):
        pc = per_core[c]
        cleanup_list = pc.get("cleanup", [])
        flat = np.full(off16 * 16, DUMP, dtype=np.int16)
        pos16 = 0
        for (q, spec, n_sub, n_sub16) in call_fill:
            seg = np.full(n_sub16, DUMP, dtype=np.int16)
            kind, a, lo = spec
            if kind == "r":
                rounds = pc["rounds"][q]
                ch = rounds[a] if a < len(rounds) else None
            else:
                ch = cleanup_list[a] if a < len(cleanup_list) else None
                lo = 0
            if ch is not None:
                npart = min(max(len(ch) - lo, 0), n_sub)
                if npart > 0:
                    part = ch[lo:lo + npart]
                    seg[:npart] = np.where(part >= 0, part,
                                           DUMP).astype(np.int16)
                    stats_real += int((part >= 0).sum())
            if c == 0:
                stats_slots += n_sub
            flat[pos16 * 16:pos16 * 16 + n_sub16] = seg
            pos16 += n_sub16 // 16
        gidx_cores.append(_wrap16(flat))
    plan["stats"] = (stats_real, stats_slots)

    # hT per (core, quarter): degree-sorted, padded, bf16, [2, 128, GPAD]
    hT_cores = []
    odeg_cores = []
    for c in range(NCORES):
        pc = per_core[c]
        hts = np.zeros((NQ, 2, P, GPAD), dtype=ml_dtypes.bfloat16)
        odt = np.zeros((NQ, P, NG), dtype=np.float32)
        hl = h[c * G:(c + 1) * G]          # [12500, 256]
        odl = out_deg[c * G:(c + 1) * G]
        for q in range(NQ):
            order = pc["orders"][q]
            hs = np.zeros((GPAD, IN_DIM), dtype=np.float32)
            hs[:G] = hl[order]
            hts[q] = np.ascontiguousarray(
                hs.T.reshape(2, P, GPAD)).astype(ml_dtypes.bfloat16)
            od = np.ones(GPAD, dtype=np.float32)
            od[:G] = odl[order]
            odt[q] = od.reshape(NG, P).T
        hT_cores.append(hts)
        odeg_cores.append(odt)

    # L2 tiles
    ideg_cores = []
    for c in range(NCORES):
        d = np.ones(GPAD, dtype=np.float32)
        d[:G] = in_deg[c * G:(c + 1) * G]
        ideg_cores.append(np.ascontiguousarray(d.reshape(NG, P).T))
    brep = np.broadcast_to(b, (P, OUT_DIM)).astype(np.float32).copy()

    return dict(plan=plan, gidx=gidx_cores, hT=hT_cores, odeg=odeg_cores,
                ideg=ideg_cores, brep=brep,
                W=W.astype(ml_dtypes.bfloat16))


_cache = {}


def _get_programs(plan):
    if "a" not in _cache:
        _cache["a"] = build_launch_1(plan)
    if "b" not in _cache:
        _cache["b"] = build_launch_2()
    return _cache["a"], _cache["b"]


def run_launch_1(nc_a, prep):
    in_maps = [{"hT": prep["hT"][c], "W": prep["W"],
                "odeg": prep["odeg"][c], "gidx": prep["gidx"][c]}
               for c in range(NCORES)]
    res = run_bass_kernel_spmd(nc_a, in_maps, list(range(NCORES)))
    return [[np.asarray(r[f"t{q}"]) for q in range(NQ)]
            for r in res.results]


def repack_partials(ptabs):
    """ptabs[c][q]: [TROWS, 128] bf16 -> pp[r]: [8, 128, NG*64] bf16."""
    pps = []
    for r in range(NCORES):
        q, half = r // 2, r % 2
        sl = np.empty((NCORES, P, NG * OUT_DIM), dtype=ml_dtypes.bfloat16)
        for c in range(NCORES):
            t = ptabs[c][q][half * GPAD:(half + 1) * GPAD, 0:OUT_DIM]
            sl[c] = t.reshape(NG, P, OUT_DIM).transpose(1, 0, 2).reshape(
                P, NG * OUT_DIM)
        pps.append(sl)
    return pps


def run_launch_2(nc_b, prep, pps):
    in_maps = [dict(pp=pps[c], ideg=prep["ideg"][c], brep=prep["brep"])
               for c in range(NCORES)]
    res = run_bass_kernel_spmd(nc_b, in_maps, list(range(NCORES)))
    outs = []
    for c in range(NCORES):
        o = np.asarray(res.results[c]["out"]).reshape(P, NG, OUT_DIM)
        outs.append(o.transpose(1, 0, 2).reshape(GPAD, OUT_DIM)[:G])
    return np.concatenate(outs, axis=0)


def kernel(h, W, b, edges):
    prep = prepare(h, W, b, edges)
    nc_a, nc_b = _get_programs(prep["plan"])
    ptabs = run_launch_1(nc_a, prep)
    pps = repack_partials(ptabs)
    out = run_launch_2(nc_b, prep, pps)
    return out.astype(np.float32)
